# revision 8
# baseline (speedup 1.0000x reference)
"""AverageSpanExtractor Trainium2 kernel — banded-mask matmul formulation.

Math: out[n, :] = mean(seq[start_n:end_n, :]) * mask_n
    = (1/width_n) * sum_s ind(start_n <= s < end_n) * seq[s, :]

Strategy (per core; data-parallel over batch across 8 cores):
  1. Stream seq [S=2048, D=512] f32 into SBUF in 8 chunks, cast to fp16
     (gpsimd).
  2. Broadcast span starts/ends across partitions with tiny one-hot
     matmuls: st_bc[j][s, n] = start_n, en_bc[j][s, n] = end_n.
  3. For each (token block b, span tile j) build the 0/1 mask tile on DVE:
       m[s, n] = (start_n <= s) * (end_n > s)        (fp16, exact)
     with s = 128*b + partition as a per-partition scalar.
  4. out_j[n, d] = sum_b m[j,b].T @ x_b  — 128 accumulating fp16 matmuls
     into 8 PSUM banks (b-outer order keeps the PE warm and lets the
     contraction stream behind the seq load).
  5. Scale by span_mask/width on ACT (per-partition scalar), DMA out.
No DRAM round-trip, no gather: HBM traffic = 4 MiB in + 2 MiB out.
"""

import numpy as np

import concourse.bacc as bacc
import concourse.bass as bass
import concourse.tile as tile
from concourse import mybir
from concourse.bass import AP

# Problem shape (hardcoded per contract).
B, S, D, N = 8, 2048, 512, 1024
NBLK = S // 128          # 16 token blocks
NTILE = N // 128         # 8 span tiles
NCHUNK = 8               # seq load chunks (2 blocks each)
BPC = NBLK // NCHUNK     # blocks per chunk

F32 = mybir.dt.float32
I32 = mybir.dt.int32
F16 = mybir.dt.float16


def build_kernel_body(tc: tile.TileContext, seq: AP, spans: AP, maskw: AP,
                      out: AP, ctx):
    nc = tc.nc
    sbuf = ctx.enter_context(tc.tile_pool(name="sbuf", bufs=1))
    const = ctx.enter_context(tc.tile_pool(name="const", bufs=1))
    bcpool = ctx.enter_context(tc.tile_pool(name="bcpool", bufs=1))
    cpool = ctx.enter_context(tc.tile_pool(name="cpool", bufs=6))
    mpool = ctx.enter_context(tc.tile_pool(name="mpool", bufs=16))
    opool = ctx.enter_context(tc.tile_pool(name="opool", bufs=8))
    psum = ctx.enter_context(tc.tile_pool(name="psum", bufs=1, space="PSUM"))

    # ---------------- constants ----------------
    # Row selectors for the broadcast matmuls: out[p, n] = rhs[k, n] for the
    # selected k (contract dim = 2: starts on partition 0, ends on 1).
    # sel_st has 1.0 on partition row 0, sel_en on row 1 (f32 build + cast).
    sel_f = const.tile([2, 2, 128], F32, tag="sel_f")
    nc.gpsimd.memset(sel_f[:], 0.0)
    for k in range(2):
        nc.gpsimd.affine_select(
            out=sel_f[:, k, :], in_=sel_f[:, k, :],
            compare_op=mybir.AluOpType.not_equal,
            fill=1.0, base=-k, pattern=[[0, 128]], channel_multiplier=1)
    sel16 = const.tile([2, 2, 128], F16, tag="sel16")
    nc.vector.tensor_copy(sel16[:], sel_f[:])
    sel_st = sel16[:, 0, :]
    sel_en = sel16[:, 1, :]

    # s_f16[p, b] = 128*b + p (token index per partition, per block).
    s_i32 = const.tile([128, NBLK], I32, tag="s_i32")
    nc.gpsimd.iota(s_i32[:], pattern=[[128, NBLK]], base=0,
                   channel_multiplier=1)
    s_f32 = const.tile([128, NBLK], F32, tag="s_f32")
    nc.vector.tensor_copy(s_f32[:], s_i32[:])

    # ---------------- span index staging ----------------
    # s2i[0, n] = start_n, s2i[1, n] = end_n  (int32 -> fp16, exact <= 2048)
    s2i = sbuf.tile([2, N], I32, tag="s2i")
    nc.sync.dma_start(s2i[:], AP(spans.tensor, 0, [[1, 2], [2, N]]))
    s2f = sbuf.tile([2, N], F16, tag="s2f")
    nc.vector.tensor_copy(s2f[:], s2i[:])

    # per-span scale = mask / width, laid out [p, j] for span n = 128*j + p
    st_pj = sbuf.tile([128, NTILE], I32, tag="st_pj")
    en_pj = sbuf.tile([128, NTILE], I32, tag="en_pj")
    mk_pj = sbuf.tile([128, NTILE], I32, tag="mk_pj")
    nc.sync.dma_start(st_pj[:], AP(spans.tensor, 0, [[2, 128], [256, NTILE]]))
    nc.sync.dma_start(en_pj[:], AP(spans.tensor, 1, [[2, 128], [256, NTILE]]))
    nc.sync.dma_start(mk_pj[:], AP(maskw.tensor, 0, [[1, 128], [128, NTILE]]))

    w_i = sbuf.tile([128, NTILE], I32, tag="w_i")
    nc.vector.tensor_tensor(out=w_i[:], in0=en_pj[:], in1=st_pj[:],
                            op=mybir.AluOpType.subtract)
    w_f = sbuf.tile([128, NTILE], F32, tag="w_f")
    nc.vector.tensor_copy(w_f[:], w_i[:])
    r_f = sbuf.tile([128, NTILE], F32, tag="r_f")
    nc.vector.reciprocal(r_f[:], w_f[:])
    m_f = sbuf.tile([128, NTILE], F32, tag="m_f")
    nc.vector.tensor_copy(m_f[:], mk_pj[:])
    scale = sbuf.tile([128, NTILE], F32, tag="scale")
    nc.vector.tensor_tensor(out=scale[:], in0=r_f[:], in1=m_f[:],
                            op=mybir.AluOpType.mult)

    # ---------------- seq load + cast to fp16 ----------------
    xbig = sbuf.tile([128, NBLK, D], F32, tag="xbig")
    xf = sbuf.tile([128, NBLK, D], F16, tag="xf")
    for q in range(NCHUNK):
        sl = (slice(None), slice(BPC * q, BPC * (q + 1)), slice(None))
        nc.sync.dma_start(
            xbig[sl],
            seq[128 * BPC * q:128 * BPC * (q + 1), :]
            .rearrange("(j p) d -> p j d", p=128))
        nc.gpsimd.tensor_copy(xf[sl], xbig[sl])

    # ---------------- start/end broadcasts (PE one-hot) ----------------
    # st_bc[j][p, n] = start_{128j+n}; en_bc[j][p, n] = end_{128j+n}
    # The broadcast matmuls borrow columns of the 8 output PSUM banks; the
    # main accumulation's start=True clears the bank afterwards.
    pouts = [psum.tile([128, D], F32, name=f"pout{j}", tag=f"pout{j}")
             for j in range(NTILE)]
    st_bc, en_bc = [], []
    for j in range(NTILE):
        nc.tensor.matmul(out=pouts[j][:, 0:128], lhsT=sel_st[:],
                         rhs=s2f[:, 128 * j:128 * (j + 1)],
                         start=True, stop=True)
        sbc = bcpool.tile([128, 128], F16, tag=f"stbc{j}")
        nc.scalar.copy(sbc[:], pouts[j][:, 0:128])
        st_bc.append(sbc)

        nc.tensor.matmul(out=pouts[j][:, 128:256], lhsT=sel_en[:],
                         rhs=s2f[:, 128 * j:128 * (j + 1)],
                         start=True, stop=True)
        ebc = bcpool.tile([128, 128], F16, tag=f"enbc{j}")
        nc.scalar.copy(ebc[:], pouts[j][:, 128:256])
        en_bc.append(ebc)

    # ---------------- main: masks on DVE, matmuls on PE ----------------
    for b in range(NBLK):
        s_col = s_f32[:, b:b + 1]
        for j in range(NTILE):
            c1 = cpool.tile([128, 128], F16, tag="c1")
            nc.vector.tensor_scalar(out=c1[:], in0=st_bc[j][:],
                                    scalar1=s_col, scalar2=None,
                                    op0=mybir.AluOpType.is_le)
            c2 = cpool.tile([128, 128], F16, tag="c2")
            nc.vector.tensor_scalar(out=c2[:], in0=en_bc[j][:],
                                    scalar1=s_col, scalar2=None,
                                    op0=mybir.AluOpType.is_gt)
            m = mpool.tile([128, 128], F16, tag="m")
            nc.vector.tensor_tensor(out=m[:], in0=c1[:], in1=c2[:],
                                    op=mybir.AluOpType.mult)
            nc.tensor.matmul(out=pouts[j][:], lhsT=m[:], rhs=xf[:, b, :],
                             start=(b == 0), stop=(b == NBLK - 1))

    # ---------------- drain: scale + store ----------------
    for j in range(NTILE):
        o_t = opool.tile([128, D], F32, tag="o")
        nc.scalar.mul(o_t[:], pouts[j][:], scale[:, j:j + 1])
        nc.sync.dma_start(out[128 * j:128 * (j + 1), :], o_t[:])


def build_nc():
    nc = bacc.Bacc("TRN2", target_bir_lowering=False, debug=False)
    seq = nc.dram_tensor("seq", [S, D], F32, kind="ExternalInput")
    spans = nc.dram_tensor("spans", [N, 2], I32, kind="ExternalInput")
    maskw = nc.dram_tensor("maskw", [N], I32, kind="ExternalInput")
    out = nc.dram_tensor("out", [N, D], F32, kind="ExternalOutput")
    from contextlib import ExitStack
    with tile.TileContext(nc) as tc:
        with ExitStack() as ctx:
            build_kernel_body(tc, seq.ap(), spans.ap(), maskw.ap(), out.ap(),
                              ctx)
    nc.compile()
    return nc


_NC_CACHE = None


def kernel(sequence_tensor: np.ndarray, span_indices: np.ndarray,
           span_indices_mask: np.ndarray) -> np.ndarray:
    global _NC_CACHE
    from concourse.bass_utils import run_bass_kernel_spmd

    if _NC_CACHE is None:
        _NC_CACHE = build_nc()
    nc = _NC_CACHE

    spans_i32 = np.ascontiguousarray(np.asarray(span_indices).astype(np.int32))
    mask_i32 = np.ascontiguousarray(np.asarray(span_indices_mask).astype(np.int32))
    seq_f32 = np.ascontiguousarray(sequence_tensor, dtype=np.float32)

    in_maps = [
        {"seq": seq_f32[b], "spans": spans_i32[b], "maskw": mask_i32[b]}
        for b in range(B)
    ]
    res = run_bass_kernel_spmd(nc, in_maps, core_ids=list(range(B)))
    return np.stack([r["out"] for r in res.results], axis=0)


# revision 14
# speedup vs baseline: 1.4796x; 1.4796x over previous
"""AverageSpanExtractor Trainium2 kernel.

Math: out[b, n, :] = mean(seq[b, start_n:end_n, :]) * mask[b, n]

Strategy (per core; data-parallel over batch across 8 cores):
  1. Load seq [S=2048, D=512] f32 into SBUF.
  2. Build exclusive prefix-sum table E[2049, 512] in DRAM:
       - per 128-token block: in-block inclusive cumsum via PE matmul with an
         upper-triangular ones matrix (U.T @ X), plus the block offset
         broadcast into the same PSUM accumulation via a one-hot selector
         matmul against a [16, 512] table of running block offsets.
       - block offsets come from block totals (all-ones matmul, one row
         copied per block) run through a tiny strict-upper-triangular matmul.
  3. Gather E[end_n] and E[start_n] rows for all 1024 spans with
     gpsimd.dma_gather (2048 rows x 2KiB), subtract on DVE, scale by
     mask/width on ACT, store.
"""

import numpy as np

import concourse.bacc as bacc
import concourse.bass as bass
import concourse.tile as tile
from concourse import mybir
from concourse.bass import AP
from concourse.library_config import mlp
from concourse.masks import make_upper_triangular
from concourse.tile_rust import add_dep_helper

# Problem shape (hardcoded per contract).
B, S, D, N = 8, 2048, 512, 1024
NBLK = S // 128          # 16 token blocks
NTILE = N // 128         # 8 span tiles
NGATHER = 4              # gather instructions (2 span tiles each)
TBL_ROWS = S + 1         # 2049

F32 = mybir.dt.float32
F32R = mybir.dt.float32r
I32 = mybir.dt.int32
I16 = mybir.dt.int16

BF16 = mybir.dt.bfloat16
F16 = mybir.dt.float16

# Matmuls run in 16-bit: the sequence data as fp16 (11 mantissa bits), the
# block-offset path as an exact bf16 hi+lo pair. 16-bit is the only matmul
# path that runs at 1 cycle/row AND warms the PE clock gate (fp32/fp32r go
# through the transpose-mode path the HAM activity monitor ignores, pinning
# the PE at 1.2 GHz).


def _mm(ap: AP, dt) -> AP:
    return ap.bitcast(dt) if ap.dtype != dt else ap


def build_kernel_body(tc: tile.TileContext, seq: AP, spans: AP, maskw: AP,
                      out: AP, ctx, dbg=None):
    nc = tc.nc
    sbuf = ctx.enter_context(tc.tile_pool(name="sbuf", bufs=1))
    const = ctx.enter_context(tc.tile_pool(name="const", bufs=1))
    epool = ctx.enter_context(tc.tile_pool(name="epool", bufs=5))
    gpool = ctx.enter_context(tc.tile_pool(name="gpool", bufs=1))
    dpool = ctx.enter_context(tc.tile_pool(name="dpool", bufs=3))
    opool = ctx.enter_context(tc.tile_pool(name="opool", bufs=3))
    psum_tot = ctx.enter_context(tc.tile_pool(name="ptot", bufs=3, space="PSUM"))
    psum_e = ctx.enter_context(tc.tile_pool(name="pe", bufs=4, space="PSUM"))
    psum_off = ctx.enter_context(tc.tile_pool(name="poff", bufs=1, space="PSUM"))
    dram = ctx.enter_context(tc.tile_pool(name="dram", bufs=1, space="DRAM"))

    table = dram.tile([TBL_ROWS, D], F32)

    # ---------------- constants (first: DVE casts lead the queue) ----------------
    # Build in f32 (memset/affine_select), then cast on DVE (0/1 exact).
    u_tri_f = const.tile([128, 128], F32, tag="u_tri_f")
    make_upper_triangular(nc, u_tri_f[:], val=1.0, diag=True)
    u_tri = const.tile([128, 128], F16, tag="u_tri")
    nc.vector.tensor_copy(u_tri[:], u_tri_f[:])

    u16s_f = const.tile([16, 16], F32, tag="u16s_f")
    make_upper_triangular(nc, u16s_f[:], val=1.0, diag=False)
    u16s = const.tile([16, 16], BF16, tag="u16s")
    nc.vector.tensor_copy(u16s[:], u16s_f[:])

    zrow = const.tile([1, D], F32, tag="zrow")
    nc.gpsimd.memset(zrow[:], 0.0)
    zrow_store = nc.sync.dma_start(table[0:1, :], zrow[:])



    # ---------------- index / scale staging (Sync queue, before loads) -------------
    # spans int32 [N, 2] = (start, end);  maskw int32 [N]
    #
    # Gather index list (linear order i within gather t of 512 idxs):
    #   i in [0,256):   end of span 256t + i
    #   i in [256,512): start of span 256t + (i - 256)
    # dma_gather reads idxs[p, c] = list[c*16 + p%16], so list position i sits
    # at column i//16, partition i%16 -> global column c = 32t + i//16.
    a32 = sbuf.tile([16, 128], I32, tag="a32")
    for t in range(NGATHER):
        # ends: dst cols 32t+u (u<16); src element = spans[256t + 16u + p, 1]
        nc.sync.dma_start(
            a32[:, 32 * t:32 * t + 16],
            AP(spans.tensor, 512 * t + 1, [[2, 16], [32, 16]]))
        # starts: dst cols 32t+16+u; src = spans[256t + 16u + p, 0]
        nc.sync.dma_start(
            a32[:, 32 * t + 16:32 * t + 32],
            AP(spans.tensor, 512 * t, [[2, 16], [32, 16]]))


    idx16 = sbuf.tile([128, 128], I16, tag="idx16")
    nc.vector.tensor_copy(idx16[0:16, :], a32[:])
    # replicate 16-partition wrap across all 128 partitions (8 Q7 cores)
    nc.scalar.dma_start(idx16[16:32, :], idx16[0:16, :])
    nc.scalar.dma_start(idx16[32:64, :], idx16[0:32, :])
    nc.scalar.dma_start(idx16[64:128, :], idx16[0:64, :])

    # ------- phase 1a: seq loads first (Sync HWDGE), cast to fp16 on DVE ---
    # fp16 keeps 11 mantissa bits (~2.4e-4 per-term); every sum reads the
    # SAME fp16 values so prefix differences stay consistent (~1e-4 relative
    # on the means).
    xbig = sbuf.tile([128, NBLK, D], F32, tag="xbig")
    xf = sbuf.tile([128, NBLK, D], F16, tag="xf")
    for q in range(NBLK // 4):
        sl = (slice(None), slice(4 * q, 4 * q + 4), slice(None))
        nc.sync.dma_start(
            xbig[sl],
            seq[512 * q:512 * (q + 1), :].rearrange("(j p) d -> p j d", p=128))
        nc.vector.tensor_copy(xf[sl], xbig[sl])

    # ------- prepare gather descriptors early (idle Q7), trigger later -----
    # Traced BEFORE any table store so the preps carry no RAW dep on the
    # table; the trigger gets explicit deps on the stores instead.
    # load the gather ucode library now: the reload blocks the Pool engine
    # ~10us, so it runs after the constants the first matmuls depend on.
    nc.gpsimd.load_library(mlp)
    gsems = [ctx.enter_context(nc.semaphore(f"gsem{t}"))
             for t in range(NGATHER)]
    gts = []
    for t in range(NGATHER):
        g_t = gpool.tile([128, 4, D], F32, tag=f"g{t}")
        nc.gpsimd.dma_gather(
            out_ap=g_t[:], in_ap=table[:], idxs_ap=idx16[:, 32 * t:32 * t + 32],
            num_idxs=512, num_idxs_reg=512, elem_size=D,
            prepare_only=True, sem=gsems[t])
        gts.append(g_t)

    # sel64[b]: [64, 128] with ones at rows k==b and k==32+b — selects the
    # bf16 hi (partitions 0:16) and lo (partitions 32:48) offset rows of o2
    # and broadcasts their sum across all 128 output partitions.
    sels = []
    for b in range(NBLK):
        sel_f = const.tile([64, 128], F32, tag=f"self{b}")
        nc.gpsimd.memset(sel_f[:], 0.0)
        nc.gpsimd.affine_select(
            out=sel_f[:], in_=sel_f[:], compare_op=mybir.AluOpType.not_equal,
            fill=1.0, base=-b, pattern=[[0, 128]], channel_multiplier=1)
        nc.gpsimd.affine_select(
            out=sel_f[:], in_=sel_f[:], compare_op=mybir.AluOpType.not_equal,
            fill=1.0, base=-(32 + b), pattern=[[0, 128]], channel_multiplier=1)
        sel_b = const.tile([64, 128], BF16, tag=f"selb{b}")
        nc.vector.tensor_copy(sel_b[:], sel_f[:])
        sels.append(sel_b)


    # per-span scale = mask / width, laid out [p, j] for span n = j*128 + p
    st_pj = sbuf.tile([128, NTILE], I32, tag="st_pj")
    en_pj = sbuf.tile([128, NTILE], I32, tag="en_pj")
    mk_pj = sbuf.tile([128, NTILE], I32, tag="mk_pj")
    nc.sync.dma_start(st_pj[:], AP(spans.tensor, 0, [[2, 128], [256, NTILE]]))
    nc.sync.dma_start(en_pj[:], AP(spans.tensor, 1, [[2, 128], [256, NTILE]]))
    nc.sync.dma_start(mk_pj[:], AP(maskw.tensor, 0, [[1, 128], [128, NTILE]]))

    w_i = sbuf.tile([128, NTILE], I32, tag="w_i")
    nc.vector.tensor_tensor(out=w_i[:], in0=en_pj[:], in1=st_pj[:],
                            op=mybir.AluOpType.subtract)
    w_f = sbuf.tile([128, NTILE], F32, tag="w_f")
    nc.vector.tensor_copy(w_f[:], w_i[:])
    r_f = sbuf.tile([128, NTILE], F32, tag="r_f")
    nc.vector.reciprocal(r_f[:], w_f[:])
    m_f = sbuf.tile([128, NTILE], F32, tag="m_f")
    nc.vector.tensor_copy(m_f[:], mk_pj[:])
    scale = sbuf.tile([128, NTILE], F32, tag="scale")
    nc.vector.tensor_tensor(out=scale[:], in0=r_f[:], in1=m_f[:],
                            op=mybir.AluOpType.mult)

    # ---------------- phase 1: in-block cumsums (no offsets yet) ----------
    # L_b = u_tri.T @ xf_b; its last row (partition 127) IS the block total,
    # so no separate totals pass is needed.
    lbig = sbuf.tile([128, NBLK, D], F32, tag="lbig")
    for b in range(NBLK):
        pl = psum_e.tile([128, D], F32, tag="pe")
        nc.tensor.matmul(out=pl[:], lhsT=u_tri[:],
                         rhs=xf[:, b, :], start=True, stop=True)
        nc.scalar.copy(lbig[:, b, :], pl[:])

    # T[16, 512] <- block totals (lbig partition 127, slots 0..14), then
    # running offsets Off = strict_upper(U16).T @ T.
    # NB: keep the source AP's partition dim honest (partition 127 only) —
    # reshaping free extents into the AP's partition slot confuses Tile's
    # dep tracking and the DMA races ahead of the producers.
    t16 = sbuf.tile([16, D], F32, tag="t16")
    nc.vector.memset(t16[:], 0.0)
    nc.sync.dma_start(t16[0:NBLK - 1, :], lbig[127:128, 0:NBLK - 1, :])

    # split t16 into bf16 hi/lo, run the tiny strict-upper matmul in bf16,
    # then pack the offsets as bf16 hi/lo into o2 rows 0:16 / 32:48.
    th = sbuf.tile([16, D], BF16, tag="th")
    nc.vector.tensor_copy(th[:], t16[:])
    tl = sbuf.tile([16, D], BF16, tag="tl")
    nc.vector.tensor_tensor(out=tl[:], in0=t16[:], in1=th[:],
                            op=mybir.AluOpType.subtract)
    poff = psum_off.tile([16, D], F32, tag="poff")
    nc.tensor.matmul(out=poff[:], lhsT=u16s[:], rhs=th[:], start=True, stop=False)
    nc.tensor.matmul(out=poff[:], lhsT=u16s[:], rhs=tl[:], start=False, stop=True)
    off16 = sbuf.tile([16, D], F32, tag="off16")
    nc.vector.tensor_copy(off16[:], poff[:])
    o2 = sbuf.tile([64, D], BF16, tag="o2")
    nc.vector.memset(o2[:], 0.0)
    nc.vector.tensor_copy(o2[0:16, :], off16[:])
    nc.vector.tensor_tensor(out=o2[32:48, :], in0=off16[:], in1=o2[0:16, :],
                            op=mybir.AluOpType.subtract)

    # ---------------- phase 1b: add offsets, store table -------------------
    store_insts = []
    for b in range(NBLK):
        p2 = psum_tot.tile([128, D], F32, tag="p2")
        nc.tensor.matmul(out=p2[:], lhsT=sels[b][:], rhs=o2[:],
                         start=True, stop=True)
        e_t = epool.tile([128, D], F32, tag="e")
        nc.vector.tensor_tensor(out=e_t[:], in0=lbig[:, b, :], in1=p2[:],
                                op=mybir.AluOpType.add)
        store_insts.append(
            nc.sync.dma_start(table[1 + 128 * b:1 + 128 * (b + 1), :], e_t[:]))
        if dbg is not None:
            nc.sync.dma_start(dbg["tbl"][1 + 128 * b:1 + 128 * (b + 1), :], e_t[:])

    if dbg is not None:
        nc.sync.dma_start(dbg["tbl"][0:1, :], zrow[:])
        nc.sync.dma_start(dbg["idx"][:], idx16[:])
        nc.sync.dma_start(dbg["scale"][:], scale[:])
        nc.sync.dma_start(dbg["xbig"][:], xbig[:])
        nc.sync.dma_start(dbg["t16"][:], t16[:])
        nc.sync.dma_start(dbg["off16"][:], off16[:])

    # ---------------- phase 2: fire prepared gathers, combine --------------
    trig = nc.gpsimd.trigger_dma(count=None)
    for st in store_insts:
        add_dep_helper(trig.ins, st.ins, sync=True, reason="gather transfers read table")
    add_dep_helper(trig.ins, zrow_store.ins, sync=True, reason="gather reads table row 0")

    for t in range(NGATHER):
        g_t = gts[t]
        if dbg is not None:
            gd = nc.sync.dma_start(dbg["g"][:, 4 * t:4 * t + 4, :], g_t[:])
            gd._wait_ge(gsems[t], 16)
            add_dep_helper(gd.ins, trig.ins, sync=False,
                           reason="consume after trigger")
        for k in range(2):
            j = 2 * t + k
            d_t = dpool.tile([128, D], F32, tag="d")
            tt = nc.vector.tensor_tensor(out=d_t[:], in0=g_t[:, k, :],
                                         in1=g_t[:, 2 + k, :],
                                         op=mybir.AluOpType.subtract)
            tt._wait_ge(gsems[t], 16)
            add_dep_helper(tt.ins, trig.ins, sync=False,
                           reason="consume after trigger")
            o_t = opool.tile([128, D], F32, tag="o")
            nc.scalar.mul(o_t[:], d_t[:], scale[:, j:j + 1])
            nc.sync.dma_start(out[128 * j:128 * (j + 1), :], o_t[:])


def build_nc(debug_taps=False):
    nc = bacc.Bacc("TRN2", target_bir_lowering=False, debug=False,
                   dynamic_dma_scratch_size=2 ** 16)
    seq = nc.dram_tensor("seq", [S, D], F32, kind="ExternalInput")
    spans = nc.dram_tensor("spans", [N, 2], I32, kind="ExternalInput")
    maskw = nc.dram_tensor("maskw", [N], I32, kind="ExternalInput")
    out = nc.dram_tensor("out", [N, D], F32, kind="ExternalOutput")
    dbg = None
    if debug_taps:
        dbg = {
            "tbl": nc.dram_tensor("dbg_tbl", [TBL_ROWS, D], F32,
                                  kind="ExternalOutput").ap(),
            "idx": nc.dram_tensor("dbg_idx", [128, 128], I16,
                                  kind="ExternalOutput").ap(),
            "scale": nc.dram_tensor("dbg_scale", [128, NTILE], F32,
                                    kind="ExternalOutput").ap(),
            "g": nc.dram_tensor("dbg_g", [128, 4 * NGATHER, D], F32,
                                kind="ExternalOutput").ap(),
            "xbig": nc.dram_tensor("dbg_xbig", [128, NBLK, D], F32,
                                   kind="ExternalOutput").ap(),
            "t16": nc.dram_tensor("dbg_t16", [16, D], F32,
                                  kind="ExternalOutput").ap(),
            "off16": nc.dram_tensor("dbg_off16", [16, D], F32,
                                    kind="ExternalOutput").ap(),
        }
    from contextlib import ExitStack
    with tile.TileContext(nc) as tc:
        with ExitStack() as ctx:
            build_kernel_body(tc, seq.ap(), spans.ap(), maskw.ap(), out.ap(),
                              ctx, dbg=dbg)
    nc.compile()
    return nc


_NC_CACHE = None


def kernel(sequence_tensor: np.ndarray, span_indices: np.ndarray,
           span_indices_mask: np.ndarray) -> np.ndarray:
    global _NC_CACHE
    from concourse.bass_utils import run_bass_kernel_spmd

    if _NC_CACHE is None:
        _NC_CACHE = build_nc()
    nc = _NC_CACHE

    spans_i32 = np.ascontiguousarray(np.asarray(span_indices).astype(np.int32))
    mask_i32 = np.ascontiguousarray(np.asarray(span_indices_mask).astype(np.int32))
    seq_f32 = np.ascontiguousarray(sequence_tensor, dtype=np.float32)

    in_maps = [
        {"seq": seq_f32[b], "spans": spans_i32[b], "maskw": mask_i32[b]}
        for b in range(B)
    ]
    res = run_bass_kernel_spmd(nc, in_maps, core_ids=list(range(B)))
    return np.stack([r["out"] for r in res.results], axis=0)


# revision 16
# speedup vs baseline: 2.2408x; 1.5145x over previous
"""AverageSpanExtractor Trainium2 kernel — banded-mask matmul formulation.

Math: out[n, :] = mean(seq[start_n:end_n, :]) * mask_n
    = (1/width_n) * sum_s ind(start_n <= s < end_n) * seq[s, :]

Strategy (per core; data-parallel over batch across 8 cores):
  1. Stream seq [S=2048, D=512] f32 into SBUF in 8 chunks, cast to fp16
     on the Scalar engine.
  2. Broadcast span starts/ends across partitions with tiny one-hot
     matmuls into borrowed PSUM columns (f16 copies on ACT).
  3. Per token block b build the 0/1 mask for ALL 1024 spans in two wide
     DVE ops (f16, exact small-int compares):
       B[p, n] = (end_n > s)            tensor_scalar is_gt, s = 128b+p
       m[p, n] = select(start_n < s+1, B, 0)   tensor_mask
  4. out_j[n, d] = sum_b m_b[:, 128j:].T @ x_b — 128 accumulating fp16
     matmuls into 8 PSUM banks on a skewed (j-staggered) schedule so
     early span tiles finish and drain while later ones still
     accumulate. A warmup burst keeps the PE HAM clock-gate at 8/8.
  5. Scale by span_mask/width on ACT (per-partition scalar), DMA out.
HBM traffic = 4 MiB in + 2 MiB out; no DRAM table, no gather.
"""

import numpy as np

import concourse.bacc as bacc
import concourse.bass as bass
import concourse.tile as tile
from concourse import mybir
from concourse.bass import AP

# Problem shape (hardcoded per contract).
B, S, D, N = 8, 2048, 512, 1024
NBLK = S // 128          # 16 token blocks
NTILE = N // 128         # 8 span tiles
NCHUNK = 8               # seq load chunks (2 blocks each)
BPC = NBLK // NCHUNK     # blocks per chunk
NWARM = 32               # PE warmup matmuls

F32 = mybir.dt.float32
I32 = mybir.dt.int32
F16 = mybir.dt.float16


def build_kernel_body(tc: tile.TileContext, seq: AP, spans: AP, maskw: AP,
                      out: AP, ctx):
    nc = tc.nc
    sbuf = ctx.enter_context(tc.tile_pool(name="sbuf", bufs=1))
    const = ctx.enter_context(tc.tile_pool(name="const", bufs=1))
    bpool = ctx.enter_context(tc.tile_pool(name="bpool", bufs=3))
    mpool = ctx.enter_context(tc.tile_pool(name="mpool", bufs=10))
    opool = ctx.enter_context(tc.tile_pool(name="opool", bufs=8))
    psum = ctx.enter_context(tc.tile_pool(name="psum", bufs=1, space="PSUM"))

    pouts = [psum.tile([128, D], F32, name=f"pout{j}", tag=f"pout{j}")
             for j in range(NTILE)]

    # ---------------- PE warmup (borrow pout0; start=True later clears) ----
    wconst = const.tile([128, 128], F16, tag="wconst")
    nc.gpsimd.memset(wconst[:], 0.0)
    for k in range(NWARM):
        nc.tensor.matmul(out=pouts[0][:, 256:384], lhsT=wconst[:],
                         rhs=wconst[:], start=True, stop=True)

    # ---------------- constants ----------------
    # Row selectors: sel16[:, 0, :] picks partition-0 row (starts),
    # sel16[:, 1, :] picks partition-1 row (ends) of the [2, N] span tile.
    sel_f = const.tile([2, 2, 128], F32, tag="sel_f")
    nc.gpsimd.memset(sel_f[:], 0.0)
    for k in range(2):
        nc.gpsimd.affine_select(
            out=sel_f[:, k, :], in_=sel_f[:, k, :],
            compare_op=mybir.AluOpType.not_equal,
            fill=1.0, base=-k, pattern=[[0, 128]], channel_multiplier=1)
    sel16 = const.tile([2, 2, 128], F16, tag="sel16")
    nc.vector.tensor_copy(sel16[:], sel_f[:])

    # s_f32[p, b] = 128*b + p; sp1_f32 = that + 1
    s_i32 = const.tile([128, NBLK], I32, tag="s_i32")
    nc.gpsimd.iota(s_i32[:], pattern=[[128, NBLK]], base=0,
                   channel_multiplier=1)
    s_f32 = const.tile([128, NBLK], F32, tag="s_f32")
    nc.vector.tensor_copy(s_f32[:], s_i32[:])
    sp1_f32 = const.tile([128, NBLK], F32, tag="sp1_f32")
    nc.vector.tensor_scalar(out=sp1_f32[:], in0=s_f32[:], scalar1=1.0,
                            scalar2=None, op0=mybir.AluOpType.add)

    # ---------------- span staging ----------------
    # s2i[0, n] = start_n, s2i[1, n] = end_n  -> f16 (exact to 2048)
    s2i = sbuf.tile([2, N], I32, tag="s2i")
    nc.sync.dma_start(s2i[:], AP(spans.tensor, 0, [[1, 2], [2, N]]))
    s2f = sbuf.tile([2, N], F16, tag="s2f")
    nc.vector.tensor_copy(s2f[:], s2i[:])

    # Broadcast via one-hot matmuls into borrowed pout PSUM columns,
    # copy f32->f16 to SBUF on ACT.
    st_bc = sbuf.tile([128, N], F16, tag="st_bc")
    en_bc = sbuf.tile([128, N], F16, tag="en_bc")
    for j in range(NTILE):
        nc.tensor.matmul(out=pouts[j][:, 0:128], lhsT=sel16[:, 0, :],
                         rhs=s2f[:, 128 * j:128 * (j + 1)],
                         start=True, stop=True)
        nc.scalar.copy(st_bc[:, 128 * j:128 * (j + 1)], pouts[j][:, 0:128])
        nc.tensor.matmul(out=pouts[j][:, 128:256], lhsT=sel16[:, 1, :],
                         rhs=s2f[:, 128 * j:128 * (j + 1)],
                         start=True, stop=True)
        nc.scalar.copy(en_bc[:, 128 * j:128 * (j + 1)], pouts[j][:, 128:256])

    # per-span scale = mask / width, laid out [p, j] for span n = 128*j + p
    st_pj = sbuf.tile([128, NTILE], I32, tag="st_pj")
    en_pj = sbuf.tile([128, NTILE], I32, tag="en_pj")
    mk_pj = sbuf.tile([128, NTILE], I32, tag="mk_pj")
    nc.sync.dma_start(st_pj[:], AP(spans.tensor, 0, [[2, 128], [256, NTILE]]))
    nc.sync.dma_start(en_pj[:], AP(spans.tensor, 1, [[2, 128], [256, NTILE]]))
    nc.sync.dma_start(mk_pj[:], AP(maskw.tensor, 0, [[1, 128], [128, NTILE]]))

    w_i = sbuf.tile([128, NTILE], I32, tag="w_i")
    nc.vector.tensor_tensor(out=w_i[:], in0=en_pj[:], in1=st_pj[:],
                            op=mybir.AluOpType.subtract)
    w_f = sbuf.tile([128, NTILE], F32, tag="w_f")
    nc.vector.tensor_copy(w_f[:], w_i[:])
    r_f = sbuf.tile([128, NTILE], F32, tag="r_f")
    nc.vector.reciprocal(r_f[:], w_f[:])
    m_f = sbuf.tile([128, NTILE], F32, tag="m_f")
    nc.vector.tensor_copy(m_f[:], mk_pj[:])
    scale = sbuf.tile([128, NTILE], F32, tag="scale")
    nc.vector.tensor_tensor(out=scale[:], in0=r_f[:], in1=m_f[:],
                            op=mybir.AluOpType.mult)

    # ---------------- seq load + cast to fp16 (ACT) ----------------
    xbig = sbuf.tile([128, NBLK, D], F32, tag="xbig")
    xf = sbuf.tile([128, NBLK, D], F16, tag="xf")
    for q in range(NCHUNK):
        sl = (slice(None), slice(BPC * q, BPC * (q + 1)), slice(None))
        nc.sync.dma_start(
            xbig[sl],
            seq[128 * BPC * q:128 * BPC * (q + 1), :]
            .rearrange("(j p) d -> p j d", p=128))
        nc.scalar.copy(xf[sl], xbig[sl])

    # ---------------- masks (DVE, 3 wide f16 ops per block) ----------------
    masks = []
    for b in range(NBLK):
        b_t = bpool.tile([128, N], F16, tag="bt")
        nc.vector.tensor_scalar(out=b_t[:], in0=en_bc[:],
                                scalar1=s_f32[:, b:b + 1], scalar2=None,
                                op0=mybir.AluOpType.is_gt)
        c_t = bpool.tile([128, N], F16, tag="ct")
        nc.vector.tensor_scalar(out=c_t[:], in0=st_bc[:],
                                scalar1=s_f32[:, b:b + 1], scalar2=None,
                                op0=mybir.AluOpType.is_le)
        m_b = mpool.tile([128, N], F16, tag="m")
        nc.vector.tensor_tensor(out=m_b[:], in0=b_t[:], in1=c_t[:],
                                op=mybir.AluOpType.mult)
        masks.append(m_b)

    # ---------------- skewed matmul schedule + early drains ----------------
    for t in range(NBLK + NTILE - 1):
        for j in range(NTILE):
            b = t - j
            if not (0 <= b < NBLK):
                continue
            nc.tensor.matmul(out=pouts[j][:],
                             lhsT=masks[b][:, 128 * j:128 * (j + 1)],
                             rhs=xf[:, b, :],
                             start=(b == 0), stop=(b == NBLK - 1))
        j_done = t - (NBLK - 1)
        if 0 <= j_done < NTILE:
            o_t = opool.tile([128, D], F32, name=f"o{j_done}", tag="o")
            nc.scalar.mul(o_t[:], pouts[j_done][:],
                          scale[:, j_done:j_done + 1])
            nc.gpsimd.dma_start(out[128 * j_done:128 * (j_done + 1), :],
                                o_t[:])


def build_nc():
    nc = bacc.Bacc("TRN2", target_bir_lowering=False, debug=False)
    seq = nc.dram_tensor("seq", [S, D], F32, kind="ExternalInput")
    spans = nc.dram_tensor("spans", [N, 2], I32, kind="ExternalInput")
    maskw = nc.dram_tensor("maskw", [N], I32, kind="ExternalInput")
    out = nc.dram_tensor("out", [N, D], F32, kind="ExternalOutput")
    from contextlib import ExitStack
    with tile.TileContext(nc) as tc:
        with ExitStack() as ctx:
            build_kernel_body(tc, seq.ap(), spans.ap(), maskw.ap(), out.ap(),
                              ctx)
    nc.compile()
    return nc


_NC_CACHE = None


def kernel(sequence_tensor: np.ndarray, span_indices: np.ndarray,
           span_indices_mask: np.ndarray) -> np.ndarray:
    global _NC_CACHE
    from concourse.bass_utils import run_bass_kernel_spmd

    if _NC_CACHE is None:
        _NC_CACHE = build_nc()
    nc = _NC_CACHE

    spans_i32 = np.ascontiguousarray(np.asarray(span_indices).astype(np.int32))
    mask_i32 = np.ascontiguousarray(np.asarray(span_indices_mask).astype(np.int32))
    seq_f32 = np.ascontiguousarray(sequence_tensor, dtype=np.float32)

    in_maps = [
        {"seq": seq_f32[b], "spans": spans_i32[b], "maskw": mask_i32[b]}
        for b in range(B)
    ]
    res = run_bass_kernel_spmd(nc, in_maps, core_ids=list(range(B)))
    return np.stack([r["out"] for r in res.results], axis=0)
